# revision 78
# baseline (speedup 1.0000x reference)
"""Trainium2 Bass kernel for nn_MultiHeadAttention_80418967650946.

Reference computation (per batch b):
  qp/kp/vp = 1x1-conv projections of q/k/v   [64, N]
  funky head view: qh[h,m,d] = qp.reshape(4, 16*N)[d, 16m+h]  (same for kh, vh)
  scores = qh @ kh * 0.25^0.5 + bias ; attn = softmax(scores)
  x[4h+d, n] = (attn @ vh)[h, n, d] ; y = LeakyReLU(BN(Wo @ x + bo), 0.2)

Sharding: 8 cores = 4 batches x 2 query-halves (n in [0,512) or [512,1024)).
Each core computes its query-half for ALL 16 heads fully locally (no
collectives): the output conv is column-wise independent, so y[:, n-half]
only needs x[:, n-half].

Key restructurings vs the v1 kernel:
  - softmax bias handled multiplicatively: exp(s+b) = exp(s)*exp(b), with
    exp(bias) precomputed host-side in fp16 (halves the dominant HBM
    stream AND turns the f32/PSUM bias-add into an all-SBUF fp16 multiply
    that DVE runs in 2x/4x perf mode; part of the multiplies go to Pool).
  - all 2-byte tensors are fp16 (better mantissa than bf16), converted on
    host so no DMA does dtype conversion (dtype-converting DMA runs 2x
    slower).
  - K projection is a standard GEMM (2048 PE cycles instead of 16384);
    the funky d-major layout is produced by a cheap SBUF->SBUF row-gather
    DMA (the funky view is just a row-concat of the standard layout).
  - attn@V accumulates 4 heads into one [128, 512] PSUM tile at rows 32j
    (PSUM APs have no 32-partition base alignment restriction); the
    softmax reciprocal is broadcast across partitions by a tiny ones
    matmul into unused rows of the same tile, so the per-head epilogue is
    just DVE reciprocal + DVE multiply + one DMA.
  - emission is software-pipelined with a one-stage skew so PE always has
    scores work queued while attn@V waits on the exp/multiply chain.
"""
import sys

if "/opt/trn_rl_repo" not in sys.path:
    sys.path.insert(0, "/opt/trn_rl_repo")

import numpy as np
import ml_dtypes

import concourse.bass as bass
import concourse.tile as tile
from concourse import bacc, mybir
from concourse.bass_utils import run_bass_kernel_spmd
from concourse.tile_rust import add_dep_helper

F32 = mybir.dt.float32
AF = mybir.ActivationFunctionType
ALU = mybir.AluOpType
PSUM = bass.MemorySpace.PSUM
F32R = mybir.dt.float32r
BF16 = mybir.dt.bfloat16


H = 16
D = 4
HID = 256
B = 4
N = 1024
NH = 512          # per-core query positions
NCORES = 8
SCALE = float(D) ** -0.5
BN_EPS = 1e-5
NEG_SLOPE = 0.2
DEBUG_DUMP = False


def _emit(nc, tc, io):
    kb, qb, vb = io["kb"], io["qb"], io["vb"]
    ebT, wkT, wvT, wqT, woT = io["ebT"], io["wkT"], io["wvT"], io["wqT"], io["woT"]
    bnv, y, e16 = io["bnv"], io["y"], io["e16"]

    with (
        tc.tile_pool(name="persist", bufs=1) as persist,
        tc.tile_pool(name="eb", bufs=3) as bp,
        tc.tile_pool(name="exp", bufs=6) as ep,
        tc.tile_pool(name="prod", bufs=6) as pp,
        tc.tile_pool(name="sml", bufs=4) as sp,
        tc.tile_pool(name="p1", bufs=1) as p1,
        tc.tile_pool(name="ps_s", bufs=3, space=PSUM) as pss,
        tc.tile_pool(name="ps_x", bufs=2, space=PSUM) as psx,
    ):
        Kp2 = persist.tile([128, H * N], BF16, tag="Kp2")
        Qp2 = persist.tile([128, H * NH], BF16, tag="Qp2")
        Vtm = persist.tile([128, H * 8 * 5], BF16, tag="Vtm")
        x_sb = persist.tile([64, NH], F32R, tag="x_sb")
        woT_sb = persist.tile([64, HID], F32R, tag="woT_sb")
        e16_sb = persist.tile([16, 64], BF16, tag="e16_sb")

        # ---------------- phase 1: inputs + projections + BN vectors --------
        # small weight tensors first so projections can start the moment
        # the bulk q/k/v transfers land
        wk_sb = p1.tile([128, 128], BF16, tag="wk_sb")
        wv_sb = p1.tile([128, 128], BF16, tag="wv_sb")
        wq_sb = p1.tile([128, 64], BF16, tag="wq_sb")
        nc.scalar.dma_start(wk_sb[:].rearrange("p (c o) -> p c o", c=2),
                            wkT.rearrange("(c p) o -> p c o", p=128))
        nc.scalar.dma_start(wq_sb[:].rearrange("p (c o) -> p c o", c=2),
                            wqT.rearrange("(c p) o -> p c o", p=128))
        nc.scalar.dma_start(wv_sb[:].rearrange("p (c o) -> p c o", c=2),
                            wvT.rearrange("(c p) o -> p c o", p=128))
        nc.gpsimd.dma_start(woT_sb[:], woT)
        nc.scalar.dma_start(e16_sb[:], e16)
        # bnv holds host-precomputed BN affine vectors: [s(2) | t(2)]
        bn_sb = persist.tile([128, 4], F32, tag="bn_sb")
        nc.gpsimd.dma_start(bn_sb[:], bnv)

        k_sb = p1.tile([128, 2048], BF16, tag="k_sb")
        q_sb = p1.tile([128, 2048], BF16, tag="q_sb")
        v_sb = p1.tile([128, 2048], BF16, tag="v_sb")
        nc.gpsimd.dma_start(k_sb[:].rearrange("p (c m n) -> p c m n", c=2, m=2),
                            kb.rearrange("(c p) (m n) -> p c m n", p=128, m=2))
        nc.sync.dma_start(q_sb[:].rearrange("p (c m n) -> p c m n", c=2, m=2),
                          qb.rearrange("(c p) (m n) -> p c m n", p=128, m=2))
        nc.scalar.dma_start(v_sb[:].rearrange("p (c n) -> p c n", c=2),
                            vb.rearrange("(c p) n -> p c n", p=128))

        # Gate tiles gk/gq occupy the eb pool slots that eb0/eb1 will use.
        # Their writers depend on k_sb/q_sb arrival, so the bulk exp(bias)
        # transfers cannot start until the latency-critical q/k loads have
        # the DMA system to themselves.
        eb_tiles = {}
        scrg = p1.tile([1, 4], BF16, tag="scrg")
        gk = bp.tile([128, 8192], BF16, tag="eb", name="ebgk")
        nc.gpsimd.tensor_copy(gk[0:1, 0:1], k_sb[0:1, 0:1])
        nc.gpsimd.tensor_copy(scrg[0:1, 0:1], gk[0:1, 0:1])
        gq = bp.tile([128, 8192], BF16, tag="eb", name="ebgq")
        nc.gpsimd.tensor_copy(gq[0:1, 0:1], q_sb[0:1, 0:1])
        nc.gpsimd.tensor_copy(scrg[0:1, 1:2], gq[0:1, 0:1])

        # eb0/eb1 fetches: slot-gated on gk/gq readers (k/q arrival)
        for ee in (0, 1):
            ebt = bp.tile([128, 8192], BF16, tag="eb", name=f"eb{ee}")
            nc.sync.dma_start(
                ebt[:, 0:4096].rearrange("p (t n) -> p t n", t=8),
                ebT[2 * ee].rearrange("p t n -> p t n"))
            nc.gpsimd.dma_start(
                ebt[:, 4096:8192].rearrange("p (t n) -> p t n", t=8),
                ebT[2 * ee + 1].rearrange("p t n -> p t n"))
            eb_tiles[ee] = ebt


        # K projection: standard GEMM kp[64, 1024] = Wk @ k, then row-gather
        # into the funky d-major layout: Kp2[d, 1024r + n] = kp[16d + r, n].
        psk = pss.tile([128, 1024], F32, tag="ps")
        for nn2 in range(2):
            for c in range(2):
                nc.tensor.matmul(
                    psk[0:64, 512 * nn2:512 * nn2 + 512],
                    wk_sb[:, 64 * c:64 * c + 64],
                    k_sb[:, 1024 * c + 512 * nn2:1024 * c + 512 * nn2 + 512],
                    start=(c == 0), stop=(c == 1))
        kproj = p1.tile([64, 1024], BF16, tag="kproj")
        nc.vector.tensor_copy(kproj[:], psk[0:64, :])
        for r in range(16):
            eng = (nc.sync, nc.gpsimd, nc.scalar)[r % 3]
            eng.dma_start(Kp2[0:4, 1024 * r:1024 * r + 1024],
                          kproj[r:r + 49:16, :])
        # single replica row-group (rg = t%2), split across 2 queues
        nc.scalar.dma_start(Kp2[32:36, 0:8192], Kp2[0:4, 0:8192])
        nc.sync.dma_start(Kp2[32:36, 8192:16384], Kp2[0:4, 8192:16384])

        # Q projection: directly into the head-major Qp2 layout (pre-scaled).
        for b4 in range(2):
            psq = pss.tile([128, 1024], F32, tag="ps")
            for g in range(4):
                j = 4 * b4 + g
                for nn2 in range(2):
                    for c in range(2):
                        nc.tensor.matmul(
                            psq[32 * g:32 * g + 4, 512 * nn2:512 * nn2 + 512],
                            wq_sb[:, 32 * c + 4 * j:32 * c + 4 * j + 4],
                            q_sb[:, 1024 * c + 512 * nn2:1024 * c + 512 * nn2 + 512],
                            start=(c == 0), stop=(c == 1), tile_position=(0, 32 * g))
            for g in range(4):
                j = 4 * b4 + g
                srcv = psq[32 * g:32 * g + 4, :].rearrange("d (a b) -> d b a", b=16)
                dstv = Qp2[0:4, :].rearrange("d (b q) -> d b q", b=16)[:, :, 64 * j:64 * j + 64]
                nc.vector.tensor_scalar_mul(dstv[:, 0:8, :], srcv[:, 0:8, :], SCALE)
                nc.scalar.mul(dstv[:, 8:16, :], srcv[:, 8:16, :], SCALE)
        nc.scalar.dma_start(Qp2[32:36, 0:4096], Qp2[0:4, 0:4096])
        nc.sync.dma_start(Qp2[32:36, 4096:8192], Qp2[0:4, 4096:8192])

        # third gate: depends on the Qp2 replica, slot-gates eb2
        g3 = bp.tile([128, 8192], BF16, tag="eb", name="ebg3")
        nc.gpsimd.partition_broadcast(g3[0:1, 0:1], Qp2[32:33, 0:1])
        nc.gpsimd.tensor_copy(scrg[0:1, 2:3], g3[0:1, 0:1])
        # V projection into Vtm [128, (h, t, c5)] bf16:
        #   Vtm[p, 40h + 5t + 0]     = 1.0   (ones column -> softmax denom)
        #   Vtm[p, 40h + 5t + 1 + d] = vh[m = 128t + p, d]  for head h
        # Only heads 0-1 are projected in the prologue; the rest stream
        # inside the head loop (head s is projected ~8 stages before its
        # attn@V needs it), shaving ~11us off the serial prologue.
        ones_f16 = p1.tile([128, 128], BF16, tag="ones_f16")
        nc.vector.memset(ones_f16[:], 1.0)
        nc.vector.tensor_copy(
            Vtm[:].rearrange("p (h t c) -> p h t c", t=8, c=5)[:, :, :, 0],
            ones_f16[:].rearrange("p (h t) -> p h t", t=8))

        def emit_vproj(s):
            psv = psx.tile([64, 64], F32, tag="psx", name=f"psv{s}")
            for c in range(2):
                nc.tensor.matmul(
                    psv[:],
                    v_sb[:, 1024 * c + s:1024 * c + s + 1009:16],
                    wv_sb[:, 64 * c:64 * c + 64],
                    start=(c == 0), stop=(c == 1),
                )
            pv = psv[:].rearrange("r (d c2) -> r d c2", c2=16)
            dst = Vtm[:].rearrange("p (h t c) -> p h t c", t=8, c=5)
            nc.vector.tensor_copy(dst[0:64, s, :, 1:5],
                                  pv[:, :, 0:16:2].transpose([0, 2, 1]))
            nc.vector.tensor_copy(dst[64:128, s, :, 1:5],
                                  pv[:, :, 1:16:2].transpose([0, 2, 1]))

        for s in range(2):
            emit_vproj(s)

        # ---------------- phase 2: attention ----------------
        Kv = [Kp2[32 * rg:32 * rg + 4, :].rearrange("d (m s) -> d m s", s=16)
              for rg in range(2)]
        Qv = [Qp2[32 * rg:32 * rg + 4, :] for rg in range(2)]

        # unnormalized x rows (f32) and per-head softmax denominators,
        # normalized in one batched pass after the head loop
        xr_sb = persist.tile([64, NH], F32, tag="xr_sb")
        den_sb = persist.tile([16, NH], F32, tag="den_sb")

        prods = [None] * 64        # product tile per stage
        ps5s = [None] * H          # per-head attn@V psum tile

        def emit_attnv(i):
            h, u = divmod(i, 4)
            if u == 0:
                ps5s[h] = psx.tile([5, NH], F32, tag="psx", name=f"ps5_{h}")
            pr = prods[i]
            for v2 in range(2):
                t = 2 * u + v2
                nc.tensor.matmul(
                    ps5s[h][:],
                    Vtm[:, 40 * h + 5 * t:40 * h + 5 * t + 5],
                    pr[:, 512 * v2:512 * v2 + 512],
                    start=(t == 0), stop=(t == 7))
            prods[i] = None

        def emit_d5(h):
            # move the head's raw attn@V output (denom + 4 x rows) to SBUF,
            # then scatter into the batched xr/den layouts via DMA.
            d5 = sp.tile([5, NH], F32, tag="d5")
            nc.vector.tensor_copy(d5[:], ps5s[h][:])
            nc.sync.dma_start(xr_sb[4 * h:4 * h + 4, :], d5[1:5, :])
            nc.sync.dma_start(den_sb[h:h + 1, :], d5[0:1, :])
            ps5s[h] = None

        for i in range(64):
            h, u = divmod(i, 4)
            ebt = eb_tiles[h // 2]
            hb = 4096 * (h % 2)

            # scores for stage i
            ps = pss.tile([128, 1024], F32, tag="ps")
            for v2 in range(2):
                t = 2 * u + v2
                rg = 0 if h == 0 else t % 2
                nc.tensor.matmul(ps[:, 512 * v2:512 * v2 + 512],
                                 Kv[rg][:, 128 * t:128 * t + 128, h],
                                 Qv[rg][:, 512 * h:512 * h + 512],
                                 start=True, stop=True,
                                 tile_position=(32 * rg, 0))
            ex = ep.tile([128, 1024], BF16, tag="ex")
            nc.scalar.activation(ex[:], ps[:], AF.Exp)
            pr = pp.tile([128, 1024], BF16, tag="pr")
            cb = hb + 1024 * u
            if i % 4 == 0:
                # split across DVE (first half, consumed first by attn@V)
                # and Pool (second half): Pool alone is 2.1us per full tile
                # and would stall the attn@V chain every 4th stage.
                nc.vector.tensor_mul(pr[:, 0:512], ex[:, 0:512],
                                     ebt[:, cb:cb + 512])
                nc.gpsimd.tensor_mul(pr[:, 512:1024], ex[:, 512:1024],
                                     ebt[:, cb + 512:cb + 1024])
            else:
                nc.vector.tensor_mul(pr[:], ex[:], ebt[:, cb:cb + 1024])
            prods[i] = pr

            # just-in-time exp(bias) prefetch, gated by the pool-slot WAR
            # dependency (slot freed by the multiplies of the tile 4 back,
            # or by the prologue gate tiles for the first two).
            if i % 8 == 0 and 2 <= i // 8 + 1 < 8:
                ee = i // 8 + 1
                ebn = bp.tile([128, 8192], BF16, tag="eb", name=f"eb{ee}")
                nc.gpsimd.dma_start(
                    ebn[:, 0:4096].rearrange("p (t n) -> p t n", t=8),
                    ebT[2 * ee].rearrange("p t n -> p t n"))
                nc.gpsimd.dma_start(
                    ebn[:, 4096:8192].rearrange("p (t n) -> p t n", t=8),
                    ebT[2 * ee + 1].rearrange("p t n -> p t n"))
                eb_tiles[ee] = ebn

            # skewed attn@V + per-head epilogue
            if i > 0:
                emit_attnv(i - 1)
                hp, up = divmod(i - 1, 4)
                if up == 3:
                    emit_d5(hp)
            # stream the V projection for head h+2, ~8 stages ahead of use
            if u == 0 and h + 2 < H:
                emit_vproj(h + 2)

        emit_attnv(63)
        emit_d5(15)

        # batched softmax normalization: one reciprocal over all 16 head
        # denominators, broadcast 16 -> 64 rows via a selector matmul on PE
        # (E16[h, 4h+d] = 1), then one elementwise multiply.
        rd16 = sp.tile([16, NH], F32, tag="rd16")
        nc.vector.reciprocal_approx_fast(rd16[:], den_sb[:])
        rh16 = sp.tile([16, NH], BF16, tag="rh16")
        nc.vector.tensor_copy(rh16[:], rd16[:])
        psb = psx.tile([64, NH], F32, tag="psx")
        nc.tensor.matmul(psb[:], e16_sb[:], rh16[:], start=True, stop=True)
        nc.vector.tensor_mul(x_sb[:], xr_sb[:], psb[:])

        if DEBUG_DUMP:
            nc.sync.dma_start(io["dbg_k"], Kp2[0:4, :])
            nc.sync.dma_start(io["dbg_q"], Qp2[0:4, :])
            nc.sync.dma_start(io["dbg_v"], Vtm[:])
            nc.sync.dma_start(io["dbg_x"], x_sb[:].bitcast(F32))

        # ---------------- phase 3: output conv + BN + LeakyReLU ----------------
        for u in range(2):
            psy = pss.tile([128, NH], F32, tag="ps")
            nc.tensor.matmul(psy[:], woT_sb[0:64, 128 * u:128 * u + 128], x_sb[:],
                             start=True, stop=True)
            y2 = sp.tile([128, NH], F32, tag="y2")
            nc.vector.tensor_scalar(y2[:], psy[:], bn_sb[:, u:u + 1], bn_sb[:, 2 + u:3 + u],
                                    ALU.mult, ALU.add)
            yt = sp.tile([128, NH], F32, tag="yt")
            nc.vector.scalar_tensor_tensor(yt[:], y2[:], NEG_SLOPE, y2[:],
                                           ALU.mult, ALU.max)
            nc.sync.dma_start(y[128 * u:128 * u + 128, :], yt[:])


def build_program():
    nc = bacc.Bacc("TRN2", target_bir_lowering=False, debug=False)
    io = {
        "kb": nc.dram_tensor("kb", [HID, N], BF16, kind="ExternalInput").ap(),
        "qb": nc.dram_tensor("qb", [HID, N], BF16, kind="ExternalInput").ap(),
        "vb": nc.dram_tensor("vb", [HID, N], BF16, kind="ExternalInput").ap(),
        "ebT": nc.dram_tensor("ebT", [H, 128, 8, NH], BF16, kind="ExternalInput").ap(),
        "wkT": nc.dram_tensor("wkT", [HID, 64], BF16, kind="ExternalInput").ap(),
        "wvT": nc.dram_tensor("wvT", [HID, 64], BF16, kind="ExternalInput").ap(),
        "wqT": nc.dram_tensor("wqT", [HID, 32], BF16, kind="ExternalInput").ap(),
        "woT": nc.dram_tensor("woT", [64, HID], F32, kind="ExternalInput").ap(),
        "bnv": nc.dram_tensor("bnv", [128, 4], F32, kind="ExternalInput").ap(),
        "e16": nc.dram_tensor("e16", [16, 64], BF16, kind="ExternalInput").ap(),
        "y": nc.dram_tensor("y", [HID, NH], F32, kind="ExternalOutput").ap(),
    }
    if DEBUG_DUMP:
        io["dbg_k"] = nc.dram_tensor("dbg_k", [4, H * N], BF16, kind="ExternalOutput").ap()
        io["dbg_q"] = nc.dram_tensor("dbg_q", [4, H * NH], BF16, kind="ExternalOutput").ap()
        io["dbg_v"] = nc.dram_tensor("dbg_v", [128, H * 8 * 5], BF16, kind="ExternalOutput").ap()
        io["dbg_x"] = nc.dram_tensor("dbg_x", [64, NH], F32, kind="ExternalOutput").ap()
    with tile.TileContext(nc) as tc:
        _emit(nc, tc, io)
    nc.compile()
    return nc


def make_in_maps(q, k, v, attn_bias, Wq, Wk, Wv, Wo, bo, gamma, beta, run_mean, run_var):
    def f32(x):
        return np.ascontiguousarray(np.asarray(x, dtype=np.float32))

    def f16(x):
        return np.ascontiguousarray(np.asarray(x, dtype=ml_dtypes.bfloat16))

    q, k, v = f16(q), f16(k), f16(v)
    attn_bias = np.asarray(attn_bias, dtype=np.float32)
    Wq, Wk, Wv = f32(Wq), f32(Wk), f32(Wv)
    Wo, bo = f32(Wo), f32(bo)
    gamma, beta, run_mean, run_var = f32(gamma), f32(beta), f32(run_mean), f32(run_var)

    eb_full = np.exp(attn_bias)                               # [B, H, N, N]

    wkT = f16(Wk.T)
    wvT = f16(Wv.T)
    woT = f32(Wo.T)
    # host-precomputed BN affine: s = gamma*rsqrt(var+eps), t = (bo-mean)*s + beta
    s_v = gamma / np.sqrt(run_var + BN_EPS)
    t_v = (bo - run_mean) * s_v + beta
    bnv = np.concatenate([x.reshape(2, 128).T for x in (s_v, t_v)], axis=1)
    bnv = f32(bnv)
    e16 = np.zeros((16, 64), ml_dtypes.bfloat16)
    for h in range(16):
        e16[h, 4 * h:4 * h + 4] = 1.0

    in_maps = []
    for core in range(NCORES):
        b, half = divmod(core, 2)
        n0 = half * NH
        rows = np.array([16 * d + 8 * half + jl for jl in range(8) for d in range(4)])
        wqT = f16(Wq[rows, :].T)                                  # [256, 32], col = 4*jl+d
        ebt = eb_full[b, :, n0:n0 + NH, :].transpose(0, 2, 1)         # [16, 1024m, 512n]
        ebT = f16(ebt.reshape(H, 8, 128, NH).transpose(0, 2, 1, 3))   # [16, 128p, 8t, 512n]
        in_maps.append({
            "kb": k[b], "qb": q[b], "vb": v[b],
            "ebT": ebT, "wkT": wkT, "wvT": wvT, "wqT": wqT, "woT": woT,
            "bnv": bnv, "e16": e16,
        })
    return in_maps


_NC_CACHE = None


def get_nc():
    global _NC_CACHE
    if _NC_CACHE is None:
        _NC_CACHE = build_program()
    return _NC_CACHE


def kernel(**inputs):
    nc = get_nc()
    in_maps = make_in_maps(**inputs)
    res = run_bass_kernel_spmd(nc, in_maps, list(range(NCORES)))
    out = np.empty((B, HID, N), dtype=np.float32)
    for core in range(NCORES):
        b, half = divmod(core, 2)
        out[b, :, half * NH:(half + 1) * NH] = res.results[core]["y"]
    return out


# revision 79
# speedup vs baseline: 1.0308x; 1.0308x over previous
"""Trainium2 Bass kernel for nn_MultiHeadAttention_80418967650946.

Reference computation (per batch b):
  qp/kp/vp = 1x1-conv projections of q/k/v   [64, N]
  funky head view: qh[h,m,d] = qp.reshape(4, 16*N)[d, 16m+h]  (same for kh, vh)
  scores = qh @ kh * 0.25^0.5 + bias ; attn = softmax(scores)
  x[4h+d, n] = (attn @ vh)[h, n, d] ; y = LeakyReLU(BN(Wo @ x + bo), 0.2)

Sharding: 8 cores = 4 batches x 2 query-halves (n in [0,512) or [512,1024)).
Each core computes its query-half for ALL 16 heads fully locally (no
collectives): the output conv is column-wise independent, so y[:, n-half]
only needs x[:, n-half].

Key restructurings vs the v1 kernel:
  - softmax bias handled multiplicatively: exp(s+b) = exp(s)*exp(b), with
    exp(bias) precomputed host-side in fp16 (halves the dominant HBM
    stream AND turns the f32/PSUM bias-add into an all-SBUF fp16 multiply
    that DVE runs in 2x/4x perf mode; part of the multiplies go to Pool).
  - all 2-byte tensors are fp16 (better mantissa than bf16), converted on
    host so no DMA does dtype conversion (dtype-converting DMA runs 2x
    slower).
  - K projection is a standard GEMM (2048 PE cycles instead of 16384);
    the funky d-major layout is produced by a cheap SBUF->SBUF row-gather
    DMA (the funky view is just a row-concat of the standard layout).
  - attn@V accumulates 4 heads into one [128, 512] PSUM tile at rows 32j
    (PSUM APs have no 32-partition base alignment restriction); the
    softmax reciprocal is broadcast across partitions by a tiny ones
    matmul into unused rows of the same tile, so the per-head epilogue is
    just DVE reciprocal + DVE multiply + one DMA.
  - emission is software-pipelined with a one-stage skew so PE always has
    scores work queued while attn@V waits on the exp/multiply chain.
"""
import sys

if "/opt/trn_rl_repo" not in sys.path:
    sys.path.insert(0, "/opt/trn_rl_repo")

import numpy as np
import ml_dtypes

import concourse.bass as bass
import concourse.tile as tile
from concourse import bacc, mybir
from concourse.bass_utils import run_bass_kernel_spmd
from concourse.tile_rust import add_dep_helper

F32 = mybir.dt.float32
AF = mybir.ActivationFunctionType
ALU = mybir.AluOpType
PSUM = bass.MemorySpace.PSUM
F32R = mybir.dt.float32r
BF16 = mybir.dt.bfloat16


H = 16
D = 4
HID = 256
B = 4
N = 1024
NH = 512          # per-core query positions
NCORES = 8
SCALE = float(D) ** -0.5
BN_EPS = 1e-5
NEG_SLOPE = 0.2
DEBUG_DUMP = False


def _emit(nc, tc, io):
    kb, qb, vb = io["kb"], io["qb"], io["vb"]
    ebT, wkT, wvT, wqT, woT = io["ebT"], io["wkT"], io["wvT"], io["wqT"], io["woT"]
    bnv, y, e16 = io["bnv"], io["y"], io["e16"]

    with (
        tc.tile_pool(name="persist", bufs=1) as persist,
        tc.tile_pool(name="eb", bufs=3) as bp,
        tc.tile_pool(name="exp", bufs=6) as ep,
        tc.tile_pool(name="prod", bufs=6) as pp,
        tc.tile_pool(name="sml", bufs=4) as sp,
        tc.tile_pool(name="p1", bufs=1) as p1,
        tc.tile_pool(name="ps_s", bufs=3, space=PSUM) as pss,
        tc.tile_pool(name="ps_x", bufs=2, space=PSUM) as psx,
    ):
        Kp2 = persist.tile([128, H * N], BF16, tag="Kp2")
        Qp2 = persist.tile([128, H * NH], BF16, tag="Qp2")
        Vtm = persist.tile([128, H * 8 * 5], BF16, tag="Vtm")
        x_sb = persist.tile([64, NH], F32R, tag="x_sb")
        woT_sb = persist.tile([64, HID], F32R, tag="woT_sb")
        e16_sb = persist.tile([16, 64], BF16, tag="e16_sb")

        # ---------------- phase 1: inputs + projections + BN vectors --------
        # small weight tensors first so projections can start the moment
        # the bulk q/k/v transfers land
        wk_sb = p1.tile([128, 128], BF16, tag="wk_sb")
        wv_sb = p1.tile([128, 128], BF16, tag="wv_sb")
        wq_sb = p1.tile([128, 64], BF16, tag="wq_sb")
        nc.scalar.dma_start(wk_sb[:].rearrange("p (c o) -> p c o", c=2),
                            wkT.rearrange("(c p) o -> p c o", p=128))
        nc.scalar.dma_start(wq_sb[:].rearrange("p (c o) -> p c o", c=2),
                            wqT.rearrange("(c p) o -> p c o", p=128))
        nc.scalar.dma_start(wv_sb[:].rearrange("p (c o) -> p c o", c=2),
                            wvT.rearrange("(c p) o -> p c o", p=128))
        nc.gpsimd.dma_start(woT_sb[:], woT)
        nc.scalar.dma_start(e16_sb[:], e16)
        # bnv holds host-precomputed BN affine vectors: [s(2) | t(2)]
        bn_sb = persist.tile([128, 4], F32, tag="bn_sb")
        nc.gpsimd.dma_start(bn_sb[:], bnv)

        k_sb = p1.tile([128, 2048], BF16, tag="k_sb")
        q_sb = p1.tile([128, 2048], BF16, tag="q_sb")
        v_sb = p1.tile([128, 2048], BF16, tag="v_sb")
        nc.gpsimd.dma_start(k_sb[:].rearrange("p (c m n) -> p c m n", c=2, m=2),
                            kb.rearrange("(c p) (m n) -> p c m n", p=128, m=2))
        nc.sync.dma_start(q_sb[:].rearrange("p (c m n) -> p c m n", c=2, m=2),
                          qb.rearrange("(c p) (m n) -> p c m n", p=128, m=2))
        nc.scalar.dma_start(v_sb[:].rearrange("p (c n) -> p c n", c=2),
                            vb.rearrange("(c p) n -> p c n", p=128))

        # Gate tiles gk/gq occupy the eb pool slots that eb0/eb1 will use.
        # Their writers depend on k_sb/q_sb arrival, so the bulk exp(bias)
        # transfers cannot start until the latency-critical q/k loads have
        # the DMA system to themselves.
        eb_tiles = {}
        scrg = p1.tile([1, 4], BF16, tag="scrg")
        gk = bp.tile([128, 8192], BF16, tag="eb", name="ebgk")
        nc.gpsimd.tensor_copy(gk[0:1, 0:1], k_sb[0:1, 0:1])
        nc.gpsimd.tensor_copy(scrg[0:1, 0:1], gk[0:1, 0:1])
        gq = bp.tile([128, 8192], BF16, tag="eb", name="ebgq")
        nc.gpsimd.tensor_copy(gq[0:1, 0:1], q_sb[0:1, 0:1])
        nc.gpsimd.tensor_copy(scrg[0:1, 1:2], gq[0:1, 0:1])

        # K projection: standard GEMM kp[64, 1024] = Wk @ k, then row-gather
        # into the funky d-major layout: Kp2[d, 1024r + n] = kp[16d + r, n].
        psk = pss.tile([128, 1024], F32, tag="ps")
        for nn2 in range(2):
            for c in range(2):
                nc.tensor.matmul(
                    psk[0:64, 512 * nn2:512 * nn2 + 512],
                    wk_sb[:, 64 * c:64 * c + 64],
                    k_sb[:, 1024 * c + 512 * nn2:1024 * c + 512 * nn2 + 512],
                    start=(c == 0), stop=(c == 1))
        kproj = p1.tile([64, 1024], BF16, tag="kproj")
        nc.vector.tensor_copy(kproj[:], psk[0:64, :])
        for r in range(16):
            eng = (nc.sync, nc.gpsimd, nc.scalar)[r % 3]
            eng.dma_start(Kp2[0:4, 1024 * r:1024 * r + 1024],
                          kproj[r:r + 49:16, :])
        # single replica row-group (rg = t%2), split across 2 queues
        nc.scalar.dma_start(Kp2[32:36, 0:8192], Kp2[0:4, 0:8192])
        nc.sync.dma_start(Kp2[32:36, 8192:16384], Kp2[0:4, 8192:16384])

        # Q projection: directly into the head-major Qp2 layout (pre-scaled).
        for b4 in range(2):
            psq = pss.tile([128, 1024], F32, tag="ps")
            for g in range(4):
                j = 4 * b4 + g
                for nn2 in range(2):
                    for c in range(2):
                        nc.tensor.matmul(
                            psq[32 * g:32 * g + 4, 512 * nn2:512 * nn2 + 512],
                            wq_sb[:, 32 * c + 4 * j:32 * c + 4 * j + 4],
                            q_sb[:, 1024 * c + 512 * nn2:1024 * c + 512 * nn2 + 512],
                            start=(c == 0), stop=(c == 1), tile_position=(0, 32 * g))
            for g in range(4):
                j = 4 * b4 + g
                srcv = psq[32 * g:32 * g + 4, :].rearrange("d (a b) -> d b a", b=16)
                dstv = Qp2[0:4, :].rearrange("d (b q) -> d b q", b=16)[:, :, 64 * j:64 * j + 64]
                nc.vector.tensor_scalar_mul(dstv[:, 0:8, :], srcv[:, 0:8, :], SCALE)
                nc.scalar.mul(dstv[:, 8:16, :], srcv[:, 8:16, :], SCALE)
        nc.scalar.dma_start(Qp2[32:36, 0:4096], Qp2[0:4, 0:4096])
        nc.sync.dma_start(Qp2[32:36, 4096:8192], Qp2[0:4, 4096:8192])

        # third gate: depends on the Qp2 replica, slot-gates eb2
        g3 = bp.tile([128, 8192], BF16, tag="eb", name="ebg3")
        nc.gpsimd.partition_broadcast(g3[0:1, 0:1], Qp2[32:33, 0:1])
        nc.gpsimd.tensor_copy(scrg[0:1, 2:3], g3[0:1, 0:1])
        # eb0/eb1 fetches: slot-gated on gk/gq readers (k/q arrival)
        for ee in (0, 1):
            ebt = bp.tile([128, 8192], BF16, tag="eb", name=f"eb{ee}")
            nc.sync.dma_start(
                ebt[:, 0:4096].rearrange("p (t n) -> p t n", t=8),
                ebT[2 * ee].rearrange("p t n -> p t n"))
            nc.gpsimd.dma_start(
                ebt[:, 4096:8192].rearrange("p (t n) -> p t n", t=8),
                ebT[2 * ee + 1].rearrange("p t n -> p t n"))
            eb_tiles[ee] = ebt

        # V projection into Vtm [128, (h, t, c5)] bf16:
        #   Vtm[p, 40h + 5t + 0]     = 1.0   (ones column -> softmax denom)
        #   Vtm[p, 40h + 5t + 1 + d] = vh[m = 128t + p, d]  for head h
        # Only heads 0-1 are projected in the prologue; the rest stream
        # inside the head loop (head s is projected ~8 stages before its
        # attn@V needs it), shaving ~11us off the serial prologue.
        ones_f16 = p1.tile([128, 128], BF16, tag="ones_f16")
        nc.vector.memset(ones_f16[:], 1.0)
        nc.vector.tensor_copy(
            Vtm[:].rearrange("p (h t c) -> p h t c", t=8, c=5)[:, :, :, 0],
            ones_f16[:].rearrange("p (h t) -> p h t", t=8))

        def emit_vproj(s):
            psv = psx.tile([64, 64], F32, tag="psx", name=f"psv{s}")
            for c in range(2):
                nc.tensor.matmul(
                    psv[:],
                    v_sb[:, 1024 * c + s:1024 * c + s + 1009:16],
                    wv_sb[:, 64 * c:64 * c + 64],
                    start=(c == 0), stop=(c == 1),
                )
            pv = psv[:].rearrange("r (d c2) -> r d c2", c2=16)
            dst = Vtm[:].rearrange("p (h t c) -> p h t c", t=8, c=5)
            nc.vector.tensor_copy(dst[0:64, s, :, 1:5],
                                  pv[:, :, 0:16:2].transpose([0, 2, 1]))
            nc.vector.tensor_copy(dst[64:128, s, :, 1:5],
                                  pv[:, :, 1:16:2].transpose([0, 2, 1]))

        for s in range(2):
            emit_vproj(s)

        # ---------------- phase 2: attention ----------------
        Kv = [Kp2[32 * rg:32 * rg + 4, :].rearrange("d (m s) -> d m s", s=16)
              for rg in range(2)]
        Qv = [Qp2[32 * rg:32 * rg + 4, :] for rg in range(2)]

        # unnormalized x rows (f32) and per-head softmax denominators,
        # normalized in one batched pass after the head loop
        xr_sb = persist.tile([64, NH], F32, tag="xr_sb")
        den_sb = persist.tile([16, NH], F32, tag="den_sb")

        prods = [None] * 64        # product tile per stage
        ps5s = [None] * H          # per-head attn@V psum tile

        def emit_attnv(i):
            h, u = divmod(i, 4)
            if u == 0:
                ps5s[h] = psx.tile([5, NH], F32, tag="psx", name=f"ps5_{h}")
            pr = prods[i]
            for v2 in range(2):
                t = 2 * u + v2
                nc.tensor.matmul(
                    ps5s[h][:],
                    Vtm[:, 40 * h + 5 * t:40 * h + 5 * t + 5],
                    pr[:, 512 * v2:512 * v2 + 512],
                    start=(t == 0), stop=(t == 7))
            prods[i] = None

        def emit_d5(h):
            # move the head's raw attn@V output (denom + 4 x rows) to SBUF,
            # then scatter into the batched xr/den layouts via DMA.
            d5 = sp.tile([5, NH], F32, tag="d5")
            nc.vector.tensor_copy(d5[:], ps5s[h][:])
            nc.sync.dma_start(xr_sb[4 * h:4 * h + 4, :], d5[1:5, :])
            nc.sync.dma_start(den_sb[h:h + 1, :], d5[0:1, :])
            ps5s[h] = None

        for i in range(64):
            h, u = divmod(i, 4)
            ebt = eb_tiles[h // 2]
            hb = 4096 * (h % 2)

            # scores for stage i
            ps = pss.tile([128, 1024], F32, tag="ps")
            for v2 in range(2):
                t = 2 * u + v2
                rg = 0 if h == 0 else t % 2
                nc.tensor.matmul(ps[:, 512 * v2:512 * v2 + 512],
                                 Kv[rg][:, 128 * t:128 * t + 128, h],
                                 Qv[rg][:, 512 * h:512 * h + 512],
                                 start=True, stop=True,
                                 tile_position=(32 * rg, 0))
            ex = ep.tile([128, 1024], BF16, tag="ex")
            nc.scalar.activation(ex[:], ps[:], AF.Exp)
            pr = pp.tile([128, 1024], BF16, tag="pr")
            cb = hb + 1024 * u
            if i % 4 == 0:
                # split across DVE (first half, consumed first by attn@V)
                # and Pool (second half): Pool alone is 2.1us per full tile
                # and would stall the attn@V chain every 4th stage.
                nc.vector.tensor_mul(pr[:, 0:512], ex[:, 0:512],
                                     ebt[:, cb:cb + 512])
                nc.gpsimd.tensor_mul(pr[:, 512:1024], ex[:, 512:1024],
                                     ebt[:, cb + 512:cb + 1024])
            else:
                nc.vector.tensor_mul(pr[:], ex[:], ebt[:, cb:cb + 1024])
            prods[i] = pr

            # just-in-time exp(bias) prefetch, gated by the pool-slot WAR
            # dependency (slot freed by the multiplies of the tile 4 back,
            # or by the prologue gate tiles for the first two).
            if i % 8 == 0 and 2 <= i // 8 + 1 < 8:
                ee = i // 8 + 1
                ebn = bp.tile([128, 8192], BF16, tag="eb", name=f"eb{ee}")
                nc.gpsimd.dma_start(
                    ebn[:, 0:4096].rearrange("p (t n) -> p t n", t=8),
                    ebT[2 * ee].rearrange("p t n -> p t n"))
                nc.gpsimd.dma_start(
                    ebn[:, 4096:8192].rearrange("p (t n) -> p t n", t=8),
                    ebT[2 * ee + 1].rearrange("p t n -> p t n"))
                eb_tiles[ee] = ebn

            # skewed attn@V + per-head epilogue
            if i > 0:
                emit_attnv(i - 1)
                hp, up = divmod(i - 1, 4)
                if up == 3:
                    emit_d5(hp)
            # stream the V projection for head h+2, ~8 stages ahead of use
            if u == 0 and h + 2 < H:
                emit_vproj(h + 2)

        emit_attnv(63)
        emit_d5(15)

        # batched softmax normalization: one reciprocal over all 16 head
        # denominators, broadcast 16 -> 64 rows via a selector matmul on PE
        # (E16[h, 4h+d] = 1), then one elementwise multiply.
        rd16 = sp.tile([16, NH], F32, tag="rd16")
        nc.vector.reciprocal_approx_fast(rd16[:], den_sb[:])
        rh16 = sp.tile([16, NH], BF16, tag="rh16")
        nc.vector.tensor_copy(rh16[:], rd16[:])
        psb = psx.tile([64, NH], F32, tag="psx")
        nc.tensor.matmul(psb[:], e16_sb[:], rh16[:], start=True, stop=True)
        nc.vector.tensor_mul(x_sb[:], xr_sb[:], psb[:])

        if DEBUG_DUMP:
            nc.sync.dma_start(io["dbg_k"], Kp2[0:4, :])
            nc.sync.dma_start(io["dbg_q"], Qp2[0:4, :])
            nc.sync.dma_start(io["dbg_v"], Vtm[:])
            nc.sync.dma_start(io["dbg_x"], x_sb[:].bitcast(F32))

        # ---------------- phase 3: output conv + BN + LeakyReLU ----------------
        for u in range(2):
            psy = pss.tile([128, NH], F32, tag="ps")
            nc.tensor.matmul(psy[:], woT_sb[0:64, 128 * u:128 * u + 128], x_sb[:],
                             start=True, stop=True)
            y2 = sp.tile([128, NH], F32, tag="y2")
            nc.vector.tensor_scalar(y2[:], psy[:], bn_sb[:, u:u + 1], bn_sb[:, 2 + u:3 + u],
                                    ALU.mult, ALU.add)
            yt = sp.tile([128, NH], F32, tag="yt")
            nc.vector.scalar_tensor_tensor(yt[:], y2[:], NEG_SLOPE, y2[:],
                                           ALU.mult, ALU.max)
            nc.sync.dma_start(y[128 * u:128 * u + 128, :], yt[:])


def build_program():
    nc = bacc.Bacc("TRN2", target_bir_lowering=False, debug=False)
    io = {
        "kb": nc.dram_tensor("kb", [HID, N], BF16, kind="ExternalInput").ap(),
        "qb": nc.dram_tensor("qb", [HID, N], BF16, kind="ExternalInput").ap(),
        "vb": nc.dram_tensor("vb", [HID, N], BF16, kind="ExternalInput").ap(),
        "ebT": nc.dram_tensor("ebT", [H, 128, 8, NH], BF16, kind="ExternalInput").ap(),
        "wkT": nc.dram_tensor("wkT", [HID, 64], BF16, kind="ExternalInput").ap(),
        "wvT": nc.dram_tensor("wvT", [HID, 64], BF16, kind="ExternalInput").ap(),
        "wqT": nc.dram_tensor("wqT", [HID, 32], BF16, kind="ExternalInput").ap(),
        "woT": nc.dram_tensor("woT", [64, HID], F32, kind="ExternalInput").ap(),
        "bnv": nc.dram_tensor("bnv", [128, 4], F32, kind="ExternalInput").ap(),
        "e16": nc.dram_tensor("e16", [16, 64], BF16, kind="ExternalInput").ap(),
        "y": nc.dram_tensor("y", [HID, NH], F32, kind="ExternalOutput").ap(),
    }
    if DEBUG_DUMP:
        io["dbg_k"] = nc.dram_tensor("dbg_k", [4, H * N], BF16, kind="ExternalOutput").ap()
        io["dbg_q"] = nc.dram_tensor("dbg_q", [4, H * NH], BF16, kind="ExternalOutput").ap()
        io["dbg_v"] = nc.dram_tensor("dbg_v", [128, H * 8 * 5], BF16, kind="ExternalOutput").ap()
        io["dbg_x"] = nc.dram_tensor("dbg_x", [64, NH], F32, kind="ExternalOutput").ap()
    with tile.TileContext(nc) as tc:
        _emit(nc, tc, io)
    nc.compile()
    return nc


def make_in_maps(q, k, v, attn_bias, Wq, Wk, Wv, Wo, bo, gamma, beta, run_mean, run_var):
    def f32(x):
        return np.ascontiguousarray(np.asarray(x, dtype=np.float32))

    def f16(x):
        return np.ascontiguousarray(np.asarray(x, dtype=ml_dtypes.bfloat16))

    q, k, v = f16(q), f16(k), f16(v)
    attn_bias = np.asarray(attn_bias, dtype=np.float32)
    Wq, Wk, Wv = f32(Wq), f32(Wk), f32(Wv)
    Wo, bo = f32(Wo), f32(bo)
    gamma, beta, run_mean, run_var = f32(gamma), f32(beta), f32(run_mean), f32(run_var)

    eb_full = np.exp(attn_bias)                               # [B, H, N, N]

    wkT = f16(Wk.T)
    wvT = f16(Wv.T)
    woT = f32(Wo.T)
    # host-precomputed BN affine: s = gamma*rsqrt(var+eps), t = (bo-mean)*s + beta
    s_v = gamma / np.sqrt(run_var + BN_EPS)
    t_v = (bo - run_mean) * s_v + beta
    bnv = np.concatenate([x.reshape(2, 128).T for x in (s_v, t_v)], axis=1)
    bnv = f32(bnv)
    e16 = np.zeros((16, 64), ml_dtypes.bfloat16)
    for h in range(16):
        e16[h, 4 * h:4 * h + 4] = 1.0

    in_maps = []
    for core in range(NCORES):
        b, half = divmod(core, 2)
        n0 = half * NH
        rows = np.array([16 * d + 8 * half + jl for jl in range(8) for d in range(4)])
        wqT = f16(Wq[rows, :].T)                                  # [256, 32], col = 4*jl+d
        ebt = eb_full[b, :, n0:n0 + NH, :].transpose(0, 2, 1)         # [16, 1024m, 512n]
        ebT = f16(ebt.reshape(H, 8, 128, NH).transpose(0, 2, 1, 3))   # [16, 128p, 8t, 512n]
        in_maps.append({
            "kb": k[b], "qb": q[b], "vb": v[b],
            "ebT": ebT, "wkT": wkT, "wvT": wvT, "wqT": wqT, "woT": woT,
            "bnv": bnv, "e16": e16,
        })
    return in_maps


_NC_CACHE = None


def get_nc():
    global _NC_CACHE
    if _NC_CACHE is None:
        _NC_CACHE = build_program()
    return _NC_CACHE


def kernel(**inputs):
    nc = get_nc()
    in_maps = make_in_maps(**inputs)
    res = run_bass_kernel_spmd(nc, in_maps, list(range(NCORES)))
    out = np.empty((B, HID, N), dtype=np.float32)
    for core in range(NCORES):
        b, half = divmod(core, 2)
        out[b, :, half * NH:(half + 1) * NH] = res.results[core]["y"]
    return out


# revision 80
# speedup vs baseline: 1.0318x; 1.0010x over previous
"""Trainium2 Bass kernel for nn_MultiHeadAttention_80418967650946.

Reference computation (per batch b):
  qp/kp/vp = 1x1-conv projections of q/k/v   [64, N]
  funky head view: qh[h,m,d] = qp.reshape(4, 16*N)[d, 16m+h]  (same for kh, vh)
  scores = qh @ kh * 0.25^0.5 + bias ; attn = softmax(scores)
  x[4h+d, n] = (attn @ vh)[h, n, d] ; y = LeakyReLU(BN(Wo @ x + bo), 0.2)

Sharding: 8 cores = 4 batches x 2 query-halves (n in [0,512) or [512,1024)).
Each core computes its query-half for ALL 16 heads fully locally (no
collectives): the output conv is column-wise independent, so y[:, n-half]
only needs x[:, n-half].

Key restructurings vs the v1 kernel:
  - softmax bias handled multiplicatively: exp(s+b) = exp(s)*exp(b), with
    exp(bias) precomputed host-side in fp16 (halves the dominant HBM
    stream AND turns the f32/PSUM bias-add into an all-SBUF fp16 multiply
    that DVE runs in 2x/4x perf mode; part of the multiplies go to Pool).
  - all 2-byte tensors are fp16 (better mantissa than bf16), converted on
    host so no DMA does dtype conversion (dtype-converting DMA runs 2x
    slower).
  - K projection is a standard GEMM (2048 PE cycles instead of 16384);
    the funky d-major layout is produced by a cheap SBUF->SBUF row-gather
    DMA (the funky view is just a row-concat of the standard layout).
  - attn@V accumulates 4 heads into one [128, 512] PSUM tile at rows 32j
    (PSUM APs have no 32-partition base alignment restriction); the
    softmax reciprocal is broadcast across partitions by a tiny ones
    matmul into unused rows of the same tile, so the per-head epilogue is
    just DVE reciprocal + DVE multiply + one DMA.
  - emission is software-pipelined with a one-stage skew so PE always has
    scores work queued while attn@V waits on the exp/multiply chain.
"""
import sys

if "/opt/trn_rl_repo" not in sys.path:
    sys.path.insert(0, "/opt/trn_rl_repo")

import numpy as np
import ml_dtypes

import concourse.bass as bass
import concourse.tile as tile
from concourse import bacc, mybir
from concourse.bass_utils import run_bass_kernel_spmd
from concourse.tile_rust import add_dep_helper

F32 = mybir.dt.float32
AF = mybir.ActivationFunctionType
ALU = mybir.AluOpType
PSUM = bass.MemorySpace.PSUM
F32R = mybir.dt.float32r
BF16 = mybir.dt.bfloat16


H = 16
D = 4
HID = 256
B = 4
N = 1024
NH = 512          # per-core query positions
NCORES = 8
SCALE = float(D) ** -0.5
BN_EPS = 1e-5
NEG_SLOPE = 0.2
DEBUG_DUMP = False


def _emit(nc, tc, io):
    kb, qb, vb = io["kb"], io["qb"], io["vb"]
    ebT, wkT, wvT, wqT, woT = io["ebT"], io["wkT"], io["wvT"], io["wqT"], io["woT"]
    bnv, y, e16 = io["bnv"], io["y"], io["e16"]

    with (
        tc.tile_pool(name="persist", bufs=1) as persist,
        tc.tile_pool(name="eb", bufs=3) as bp,
        tc.tile_pool(name="exp", bufs=6) as ep,
        tc.tile_pool(name="prod", bufs=6) as pp,
        tc.tile_pool(name="sml", bufs=4) as sp,
        tc.tile_pool(name="p1", bufs=1) as p1,
        tc.tile_pool(name="ps_s", bufs=3, space=PSUM) as pss,
        tc.tile_pool(name="ps_x", bufs=2, space=PSUM) as psx,
    ):
        Kp2 = persist.tile([128, H * N], BF16, tag="Kp2")
        Qp2 = persist.tile([128, H * NH], BF16, tag="Qp2")
        Vtm = persist.tile([128, H * 8 * 5], BF16, tag="Vtm")
        x_sb = persist.tile([64, NH], F32R, tag="x_sb")
        woT_sb = persist.tile([64, HID], F32R, tag="woT_sb")
        e16_sb = persist.tile([16, 64], BF16, tag="e16_sb")

        # ---------------- phase 1: inputs + projections + BN vectors --------
        # small weight tensors first so projections can start the moment
        # the bulk q/k/v transfers land
        wk_sb = p1.tile([128, 128], BF16, tag="wk_sb")
        wv_sb = p1.tile([128, 128], BF16, tag="wv_sb")
        wq_sb = p1.tile([128, 64], BF16, tag="wq_sb")
        nc.scalar.dma_start(wk_sb[:].rearrange("p (c o) -> p c o", c=2),
                            wkT.rearrange("(c p) o -> p c o", p=128))
        nc.scalar.dma_start(wq_sb[:].rearrange("p (c o) -> p c o", c=2),
                            wqT.rearrange("(c p) o -> p c o", p=128))
        nc.scalar.dma_start(wv_sb[:].rearrange("p (c o) -> p c o", c=2),
                            wvT.rearrange("(c p) o -> p c o", p=128))
        nc.gpsimd.dma_start(woT_sb[:], woT)
        nc.scalar.dma_start(e16_sb[:], e16)
        # bnv holds host-precomputed BN affine vectors: [s(2) | t(2)]
        bn_sb = persist.tile([128, 4], F32, tag="bn_sb")
        nc.gpsimd.dma_start(bn_sb[:], bnv)

        k_sb = p1.tile([128, 2048], BF16, tag="k_sb")
        q_sb = p1.tile([128, 2048], BF16, tag="q_sb")
        v_sb = p1.tile([128, 2048], BF16, tag="v_sb")
        nc.gpsimd.dma_start(k_sb[:].rearrange("p (c m n) -> p c m n", c=2, m=2),
                            kb.rearrange("(c p) (m n) -> p c m n", p=128, m=2))
        nc.sync.dma_start(q_sb[:].rearrange("p (c m n) -> p c m n", c=2, m=2),
                          qb.rearrange("(c p) (m n) -> p c m n", p=128, m=2))
        nc.scalar.dma_start(v_sb[:].rearrange("p (c n) -> p c n", c=2),
                            vb.rearrange("(c p) n -> p c n", p=128))

        # Gate tiles gk/gq occupy the eb pool slots that eb0/eb1 will use.
        # Their writers depend on k_sb/q_sb arrival, so the bulk exp(bias)
        # transfers cannot start until the latency-critical q/k loads have
        # the DMA system to themselves.
        eb_tiles = {}
        scrg = p1.tile([1, 4], BF16, tag="scrg")
        gk = bp.tile([128, 8192], BF16, tag="eb", name="ebgk")
        nc.gpsimd.tensor_copy(gk[0:1, 0:1], k_sb[0:1, 0:1])
        nc.gpsimd.tensor_copy(scrg[0:1, 0:1], gk[0:1, 0:1])
        gq = bp.tile([128, 8192], BF16, tag="eb", name="ebgq")
        nc.gpsimd.tensor_copy(gq[0:1, 0:1], q_sb[0:1, 0:1])
        nc.gpsimd.tensor_copy(scrg[0:1, 1:2], gq[0:1, 0:1])

        # K projection: standard GEMM kp[64, 1024] = Wk @ k, then row-gather
        # into the funky d-major layout: Kp2[d, 1024r + n] = kp[16d + r, n].
        psk = pss.tile([128, 1024], F32, tag="ps")
        for nn2 in range(2):
            for c in range(2):
                nc.tensor.matmul(
                    psk[0:64, 512 * nn2:512 * nn2 + 512],
                    wk_sb[:, 64 * c:64 * c + 64],
                    k_sb[:, 1024 * c + 512 * nn2:1024 * c + 512 * nn2 + 512],
                    start=(c == 0), stop=(c == 1))
        kproj = p1.tile([64, 1024], BF16, tag="kproj")
        nc.vector.tensor_copy(kproj[:], psk[0:64, :])
        for r in range(16):
            # all gathers on the (prologue-idle) scalar queue: they wait on
            # kproj and would otherwise FIFO-block the eb halves and replica
            # DMAs queued behind them on sync/gpsimd
            nc.scalar.dma_start(Kp2[0:4, 1024 * r:1024 * r + 1024],
                                kproj[r:r + 49:16, :])
        # single replica row-group (rg = t%2), split across 2 queues
        nc.scalar.dma_start(Kp2[32:36, 0:8192], Kp2[0:4, 0:8192])
        nc.sync.dma_start(Kp2[32:36, 8192:16384], Kp2[0:4, 8192:16384])

        # Q projection: directly into the head-major Qp2 layout (pre-scaled).
        for b4 in range(2):
            psq = pss.tile([128, 1024], F32, tag="ps")
            for g in range(4):
                j = 4 * b4 + g
                for nn2 in range(2):
                    for c in range(2):
                        nc.tensor.matmul(
                            psq[32 * g:32 * g + 4, 512 * nn2:512 * nn2 + 512],
                            wq_sb[:, 32 * c + 4 * j:32 * c + 4 * j + 4],
                            q_sb[:, 1024 * c + 512 * nn2:1024 * c + 512 * nn2 + 512],
                            start=(c == 0), stop=(c == 1), tile_position=(0, 32 * g))
            for g in range(4):
                j = 4 * b4 + g
                srcv = psq[32 * g:32 * g + 4, :].rearrange("d (a b) -> d b a", b=16)
                dstv = Qp2[0:4, :].rearrange("d (b q) -> d b q", b=16)[:, :, 64 * j:64 * j + 64]
                nc.vector.tensor_scalar_mul(dstv[:, 0:8, :], srcv[:, 0:8, :], SCALE)
                nc.scalar.mul(dstv[:, 8:16, :], srcv[:, 8:16, :], SCALE)
        nc.scalar.dma_start(Qp2[32:36, 0:4096], Qp2[0:4, 0:4096])
        nc.sync.dma_start(Qp2[32:36, 4096:8192], Qp2[0:4, 4096:8192])

        # third gate: depends on the Qp2 replica, slot-gates eb2
        g3 = bp.tile([128, 8192], BF16, tag="eb", name="ebg3")
        nc.gpsimd.partition_broadcast(g3[0:1, 0:1], Qp2[32:33, 0:1])
        nc.gpsimd.tensor_copy(scrg[0:1, 2:3], g3[0:1, 0:1])
        # eb0/eb1 fetches: slot-gated on gk/gq readers (k/q arrival)
        for ee in (0, 1):
            ebt = bp.tile([128, 8192], BF16, tag="eb", name=f"eb{ee}")
            nc.sync.dma_start(
                ebt[:, 0:4096].rearrange("p (t n) -> p t n", t=8),
                ebT[2 * ee].rearrange("p t n -> p t n"))
            nc.gpsimd.dma_start(
                ebt[:, 4096:8192].rearrange("p (t n) -> p t n", t=8),
                ebT[2 * ee + 1].rearrange("p t n -> p t n"))
            eb_tiles[ee] = ebt

        # V projection into Vtm [128, (h, t, c5)] bf16:
        #   Vtm[p, 40h + 5t + 0]     = 1.0   (ones column -> softmax denom)
        #   Vtm[p, 40h + 5t + 1 + d] = vh[m = 128t + p, d]  for head h
        # Only heads 0-1 are projected in the prologue; the rest stream
        # inside the head loop (head s is projected ~8 stages before its
        # attn@V needs it), shaving ~11us off the serial prologue.
        ones_f16 = p1.tile([128, 128], BF16, tag="ones_f16")
        nc.vector.memset(ones_f16[:], 1.0)
        nc.vector.tensor_copy(
            Vtm[:].rearrange("p (h t c) -> p h t c", t=8, c=5)[:, :, :, 0],
            ones_f16[:].rearrange("p (h t) -> p h t", t=8))

        def emit_vproj(s):
            psv = psx.tile([64, 64], F32, tag="psx", name=f"psv{s}")
            for c in range(2):
                nc.tensor.matmul(
                    psv[:],
                    v_sb[:, 1024 * c + s:1024 * c + s + 1009:16],
                    wv_sb[:, 64 * c:64 * c + 64],
                    start=(c == 0), stop=(c == 1),
                )
            pv = psv[:].rearrange("r (d c2) -> r d c2", c2=16)
            dst = Vtm[:].rearrange("p (h t c) -> p h t c", t=8, c=5)
            nc.vector.tensor_copy(dst[0:64, s, :, 1:5],
                                  pv[:, :, 0:16:2].transpose([0, 2, 1]))
            nc.vector.tensor_copy(dst[64:128, s, :, 1:5],
                                  pv[:, :, 1:16:2].transpose([0, 2, 1]))

        for s in range(2):
            emit_vproj(s)

        # ---------------- phase 2: attention ----------------
        Kv = [Kp2[32 * rg:32 * rg + 4, :].rearrange("d (m s) -> d m s", s=16)
              for rg in range(2)]
        Qv = [Qp2[32 * rg:32 * rg + 4, :] for rg in range(2)]

        # unnormalized x rows (f32) and per-head softmax denominators,
        # normalized in one batched pass after the head loop
        xr_sb = persist.tile([64, NH], F32, tag="xr_sb")
        den_sb = persist.tile([16, NH], F32, tag="den_sb")

        prods = [None] * 64        # product tile per stage
        ps5s = [None] * H          # per-head attn@V psum tile

        def emit_attnv(i):
            h, u = divmod(i, 4)
            if u == 0:
                ps5s[h] = psx.tile([5, NH], F32, tag="psx", name=f"ps5_{h}")
            pr = prods[i]
            for v2 in range(2):
                t = 2 * u + v2
                nc.tensor.matmul(
                    ps5s[h][:],
                    Vtm[:, 40 * h + 5 * t:40 * h + 5 * t + 5],
                    pr[:, 512 * v2:512 * v2 + 512],
                    start=(t == 0), stop=(t == 7))
            prods[i] = None

        def emit_d5(h):
            # move the head's raw attn@V output (denom + 4 x rows) to SBUF,
            # then scatter into the batched xr/den layouts via DMA.
            d5 = sp.tile([5, NH], F32, tag="d5")
            nc.vector.tensor_copy(d5[:], ps5s[h][:])
            nc.sync.dma_start(xr_sb[4 * h:4 * h + 4, :], d5[1:5, :])
            nc.sync.dma_start(den_sb[h:h + 1, :], d5[0:1, :])
            ps5s[h] = None

        for i in range(64):
            h, u = divmod(i, 4)
            ebt = eb_tiles[h // 2]
            hb = 4096 * (h % 2)

            # scores for stage i
            ps = pss.tile([128, 1024], F32, tag="ps")
            for v2 in range(2):
                t = 2 * u + v2
                rg = 0 if h == 0 else t % 2
                nc.tensor.matmul(ps[:, 512 * v2:512 * v2 + 512],
                                 Kv[rg][:, 128 * t:128 * t + 128, h],
                                 Qv[rg][:, 512 * h:512 * h + 512],
                                 start=True, stop=True,
                                 tile_position=(32 * rg, 0))
            ex = ep.tile([128, 1024], BF16, tag="ex")
            nc.scalar.activation(ex[:], ps[:], AF.Exp)
            pr = pp.tile([128, 1024], BF16, tag="pr")
            cb = hb + 1024 * u
            if i % 4 == 0:
                # split across DVE (first half, consumed first by attn@V)
                # and Pool (second half): Pool alone is 2.1us per full tile
                # and would stall the attn@V chain every 4th stage.
                nc.vector.tensor_mul(pr[:, 0:512], ex[:, 0:512],
                                     ebt[:, cb:cb + 512])
                nc.gpsimd.tensor_mul(pr[:, 512:1024], ex[:, 512:1024],
                                     ebt[:, cb + 512:cb + 1024])
            else:
                nc.vector.tensor_mul(pr[:], ex[:], ebt[:, cb:cb + 1024])
            prods[i] = pr

            # just-in-time exp(bias) prefetch, gated by the pool-slot WAR
            # dependency (slot freed by the multiplies of the tile 4 back,
            # or by the prologue gate tiles for the first two).
            if i % 8 == 0 and 2 <= i // 8 + 1 < 8:
                ee = i // 8 + 1
                ebn = bp.tile([128, 8192], BF16, tag="eb", name=f"eb{ee}")
                nc.gpsimd.dma_start(
                    ebn[:, 0:4096].rearrange("p (t n) -> p t n", t=8),
                    ebT[2 * ee].rearrange("p t n -> p t n"))
                nc.gpsimd.dma_start(
                    ebn[:, 4096:8192].rearrange("p (t n) -> p t n", t=8),
                    ebT[2 * ee + 1].rearrange("p t n -> p t n"))
                eb_tiles[ee] = ebn

            # skewed attn@V + per-head epilogue
            if i > 0:
                emit_attnv(i - 1)
                hp, up = divmod(i - 1, 4)
                if up == 3:
                    emit_d5(hp)
            # stream the V projection for head h+2, ~8 stages ahead of use
            if u == 0 and h + 2 < H:
                emit_vproj(h + 2)

        emit_attnv(63)
        emit_d5(15)

        # batched softmax normalization: one reciprocal over all 16 head
        # denominators, broadcast 16 -> 64 rows via a selector matmul on PE
        # (E16[h, 4h+d] = 1), then one elementwise multiply.
        rd16 = sp.tile([16, NH], F32, tag="rd16")
        nc.vector.reciprocal_approx_fast(rd16[:], den_sb[:])
        rh16 = sp.tile([16, NH], BF16, tag="rh16")
        nc.vector.tensor_copy(rh16[:], rd16[:])
        psb = psx.tile([64, NH], F32, tag="psx")
        nc.tensor.matmul(psb[:], e16_sb[:], rh16[:], start=True, stop=True)
        nc.vector.tensor_mul(x_sb[:], xr_sb[:], psb[:])

        if DEBUG_DUMP:
            nc.sync.dma_start(io["dbg_k"], Kp2[0:4, :])
            nc.sync.dma_start(io["dbg_q"], Qp2[0:4, :])
            nc.sync.dma_start(io["dbg_v"], Vtm[:])
            nc.sync.dma_start(io["dbg_x"], x_sb[:].bitcast(F32))

        # ---------------- phase 3: output conv + BN + LeakyReLU ----------------
        for u in range(2):
            psy = pss.tile([128, NH], F32, tag="ps")
            nc.tensor.matmul(psy[:], woT_sb[0:64, 128 * u:128 * u + 128], x_sb[:],
                             start=True, stop=True)
            y2 = sp.tile([128, NH], F32, tag="y2")
            nc.vector.tensor_scalar(y2[:], psy[:], bn_sb[:, u:u + 1], bn_sb[:, 2 + u:3 + u],
                                    ALU.mult, ALU.add)
            yt = sp.tile([128, NH], F32, tag="yt")
            nc.vector.scalar_tensor_tensor(yt[:], y2[:], NEG_SLOPE, y2[:],
                                           ALU.mult, ALU.max)
            nc.sync.dma_start(y[128 * u:128 * u + 128, :], yt[:])


def build_program():
    nc = bacc.Bacc("TRN2", target_bir_lowering=False, debug=False)
    io = {
        "kb": nc.dram_tensor("kb", [HID, N], BF16, kind="ExternalInput").ap(),
        "qb": nc.dram_tensor("qb", [HID, N], BF16, kind="ExternalInput").ap(),
        "vb": nc.dram_tensor("vb", [HID, N], BF16, kind="ExternalInput").ap(),
        "ebT": nc.dram_tensor("ebT", [H, 128, 8, NH], BF16, kind="ExternalInput").ap(),
        "wkT": nc.dram_tensor("wkT", [HID, 64], BF16, kind="ExternalInput").ap(),
        "wvT": nc.dram_tensor("wvT", [HID, 64], BF16, kind="ExternalInput").ap(),
        "wqT": nc.dram_tensor("wqT", [HID, 32], BF16, kind="ExternalInput").ap(),
        "woT": nc.dram_tensor("woT", [64, HID], F32, kind="ExternalInput").ap(),
        "bnv": nc.dram_tensor("bnv", [128, 4], F32, kind="ExternalInput").ap(),
        "e16": nc.dram_tensor("e16", [16, 64], BF16, kind="ExternalInput").ap(),
        "y": nc.dram_tensor("y", [HID, NH], F32, kind="ExternalOutput").ap(),
    }
    if DEBUG_DUMP:
        io["dbg_k"] = nc.dram_tensor("dbg_k", [4, H * N], BF16, kind="ExternalOutput").ap()
        io["dbg_q"] = nc.dram_tensor("dbg_q", [4, H * NH], BF16, kind="ExternalOutput").ap()
        io["dbg_v"] = nc.dram_tensor("dbg_v", [128, H * 8 * 5], BF16, kind="ExternalOutput").ap()
        io["dbg_x"] = nc.dram_tensor("dbg_x", [64, NH], F32, kind="ExternalOutput").ap()
    with tile.TileContext(nc) as tc:
        _emit(nc, tc, io)
    nc.compile()
    return nc


def make_in_maps(q, k, v, attn_bias, Wq, Wk, Wv, Wo, bo, gamma, beta, run_mean, run_var):
    def f32(x):
        return np.ascontiguousarray(np.asarray(x, dtype=np.float32))

    def f16(x):
        return np.ascontiguousarray(np.asarray(x, dtype=ml_dtypes.bfloat16))

    q, k, v = f16(q), f16(k), f16(v)
    attn_bias = np.asarray(attn_bias, dtype=np.float32)
    Wq, Wk, Wv = f32(Wq), f32(Wk), f32(Wv)
    Wo, bo = f32(Wo), f32(bo)
    gamma, beta, run_mean, run_var = f32(gamma), f32(beta), f32(run_mean), f32(run_var)

    eb_full = np.exp(attn_bias)                               # [B, H, N, N]

    wkT = f16(Wk.T)
    wvT = f16(Wv.T)
    woT = f32(Wo.T)
    # host-precomputed BN affine: s = gamma*rsqrt(var+eps), t = (bo-mean)*s + beta
    s_v = gamma / np.sqrt(run_var + BN_EPS)
    t_v = (bo - run_mean) * s_v + beta
    bnv = np.concatenate([x.reshape(2, 128).T for x in (s_v, t_v)], axis=1)
    bnv = f32(bnv)
    e16 = np.zeros((16, 64), ml_dtypes.bfloat16)
    for h in range(16):
        e16[h, 4 * h:4 * h + 4] = 1.0

    in_maps = []
    for core in range(NCORES):
        b, half = divmod(core, 2)
        n0 = half * NH
        rows = np.array([16 * d + 8 * half + jl for jl in range(8) for d in range(4)])
        wqT = f16(Wq[rows, :].T)                                  # [256, 32], col = 4*jl+d
        ebt = eb_full[b, :, n0:n0 + NH, :].transpose(0, 2, 1)         # [16, 1024m, 512n]
        ebT = f16(ebt.reshape(H, 8, 128, NH).transpose(0, 2, 1, 3))   # [16, 128p, 8t, 512n]
        in_maps.append({
            "kb": k[b], "qb": q[b], "vb": v[b],
            "ebT": ebT, "wkT": wkT, "wvT": wvT, "wqT": wqT, "woT": woT,
            "bnv": bnv, "e16": e16,
        })
    return in_maps


_NC_CACHE = None


def get_nc():
    global _NC_CACHE
    if _NC_CACHE is None:
        _NC_CACHE = build_program()
    return _NC_CACHE


def kernel(**inputs):
    nc = get_nc()
    in_maps = make_in_maps(**inputs)
    res = run_bass_kernel_spmd(nc, in_maps, list(range(NCORES)))
    out = np.empty((B, HID, N), dtype=np.float32)
    for core in range(NCORES):
        b, half = divmod(core, 2)
        out[b, :, half * NH:(half + 1) * NH] = res.results[core]["y"]
    return out
